# revision 22
# baseline (speedup 1.0000x reference)
"""Trainium2 Bass kernel for cross-attention (cosine-normalized, 8 heads).

Reference computation (full inputs x,y [1,4096,64]):
  q = x@Wq+bq ; k,v = split(y@Wkv+bkv) ; per head (8 heads, dim 8):
  attn = softmax(l2norm(q) @ l2norm(k)^T) ; out = attn@v
  result = concat_heads(out) @ We + be

Sharding: one head per NeuronCore (8 heads / 8 cores), SPMD program with
per-core weight slices. Each core returns resT_h = (out_h @ We_h + be/8)^T
as [64, 4096]; the host sums over cores and transposes.

Device algorithm per core (head h):
  - host passes xTe=[x^T; 1] [65,4096] (ones row folds biases into matmuls),
    yTe likewise, weight slices with bias rows, and a selector constant.
  - qT [8,4096] = Wqe^T @ xTe ; kT likewise (PE, K=65, fp32r).
  - norms in free layout: squares on DVE; selector matmuls pack per-block
    column sums into one [8,512] psum; invsqrt = newton(recip(sqrt)));
    flatten via DMA, replicate via gpsimd partition_broadcast; qT/kT
    normalized into fp32r tiles.
  - v chunks [128,8] = yTe_chunk^T @ Wve stored bf16 with ones column
    (vext [128, 9*32]); the ones column produces the softmax denominator.
  - main loop (8 q-blocks x 16 chunk-groups):
      scores^T [128,1024] = kTn_chunks^T qTn_block (fp32r, two matmuls)
      expS bf16 = Exp(scores) on ScalarE (cosine scores in [-1,1]: no
        max-subtraction needed)
      po [9,512] += vext_chunk^T @ expS  (rows 0-7 numerator, row 8 denom)
  - tail: invden = recip_accurate(den); oTe (incl. denom row) scaled by
    broadcast invden -> row 8 becomes exactly 1 = bias row for the final
    fp32 projection resT = WeBe^T @ oTe; DMA out.
"""

import sys

import numpy as np

for _p in ("/opt/trn_rl_repo",):
    if _p not in sys.path:
        sys.path.insert(0, _p)

from contextlib import ExitStack

import concourse.bass as bass
import concourse.tile as tile
from concourse import bacc, mybir
from concourse.bass import ts
from concourse.bass_utils import run_bass_kernel_spmd

F32 = mybir.dt.float32
F32R = mybir.dt.float32r
BF16 = mybir.dt.bfloat16

HW = 4096          # sequence length
C = 64             # model dim
H = 8              # heads
D = 8              # head dim
CE = C + 1         # +ones row for bias folding
QB = 512           # q block
NQB = HW // QB     # 8
KC = 128           # k chunk
NKC = HW // KC     # 32
GROUPS = [3] * 10 + [2]   # k-chunks per exp/ACT group (32 total)
GMAX = max(GROUPS)
VW = D + 1         # v + ones column

REPL = "dma"        # inv replication: gpsimd partition_broadcast vs row DMAs

_BUILT = None
TRACE = False
LAST_RESULTS = None


def _body(ctx, tc, dram):
    nc = tc.nc
    xTe_d, yTe_d, wqe_d, wke_d, wve_d, webe_d, sel_d, out_d = dram
    U32 = mybir.dt.uint32

    const = ctx.enter_context(tc.tile_pool(name="const", bufs=1))
    expp = ctx.enter_context(tc.tile_pool(name="exps", bufs=3))
    ps_s = ctx.enter_context(tc.tile_pool(name="ps_s", bufs=2, space="PSUM"))
    ps_o = ctx.enter_context(tc.tile_pool(name="ps_o", bufs=2, space="PSUM"))

    # K-padded tiles: rows beyond the live ones are zeroed so every matmul
    # contracts over K=128 (K<=32 matmuls run at half the PE clock - measured).
    # All zeroing on gpsimd to keep the DVE free for the norm chain.
    xTe = const.tile([KC, HW], F32R)   # host-padded: rows CE..127 zero
    yTe = const.tile([KC, HW], F32R)
    sqq = const.tile([KC, HW], F32R)   # q squares scratch / f32r oT staging
    sqk = const.tile([KC, HW], F32R)   # k squares scratch
    qTn = const.tile([KC, HW], BF16)
    kTn = const.tile([KC, HW], BF16)
    vext = const.tile([KC, VW * NKC], BF16)
    nc.gpsimd.memset(sqq[:].bitcast(U32), 0)
    nc.gpsimd.memset(sqk[:].bitcast(U32), 0)
    nc.gpsimd.memset(qTn[:].bitcast(mybir.dt.uint16), 0)
    nc.gpsimd.memset(kTn[:].bitcast(mybir.dt.uint16), 0)
    nc.gpsimd.memset(vext[:], 1.0)

    # ---------------- loads ----------------
    wqe = const.tile([KC, D], F32R)
    nc.sync.dma_start(wqe[:], wqe_d)
    wke = const.tile([KC, D], F32R)
    nc.sync.dma_start(wke[:], wke_d)
    wve = const.tile([KC, D], F32R)
    nc.sync.dma_start(wve[:], wve_d)
    webe = const.tile([KC, C], F32R)
    nc.scalar.dma_start(webe[:], webe_d)
    sel = const.tile([KC, D * NQB], F32R)
    nc.scalar.dma_start(sel[:], sel_d)
    dmae = [nc.sync, nc.scalar]
    for j in range(NQB):
        dmae[j % 2].dma_start(xTe[:, ts(j, QB)], xTe_d[:, ts(j, QB)])
        dmae[(j + 1) % 2].dma_start(yTe[:, ts(j, QB)], yTe_d[:, ts(j, QB)])

    # ---------------- q/k projections (transposed layout) ----------------
    qT = const.tile([D, HW], F32)
    kT = const.tile([D, HW], F32)
    for dst, w, srct in ((qT, wqe, xTe), (kT, wke, yTe)):
        for j in range(NQB):
            ps = ps_s.tile([D, QB], F32, tag="pss")
            nc.tensor.matmul(ps[:], w[:], srct[:, ts(j, QB)], start=True,
                             stop=True)
            nc.vector.tensor_copy(dst[:, ts(j, QB)], ps[:])

    # ---------------- inverse norms (free layout, partition-packed) -------
    ssq_q = const.tile([NQB, QB], F32)
    ssq_k = const.tile([NQB, QB], F32)
    for srct, ssq, sq in ((qT, ssq_q, sqq), (kT, ssq_k, sqk)):
        nc.vector.tensor_mul(sq[0:D, :], srct[:], srct[:])
        ps = ps_s.tile([NQB, QB], F32, tag="pss")
        for j in range(NQB):
            nc.tensor.matmul(ps[:], sel[:, ts(j, D)], sq[:, ts(j, QB)],
                             start=(j == 0), stop=(j == NQB - 1))
        nc.vector.tensor_copy(ssq[:], ps[:])

    # invsqrt = newton(recip_accurate(sqrt(ssq))); separate scratches so the
    # q and k chains interleave
    inv_q = const.tile([NQB, QB], F32)
    inv_k = const.tile([NQB, QB], F32)
    scrq = const.tile([NQB, QB], F32)
    scr2q = const.tile([NQB, QB], F32)
    scrk = const.tile([NQB, QB], F32)
    scr2k = const.tile([NQB, QB], F32)
    for ssq, inv, sa, sb in ((ssq_q, inv_q, scrq, scr2q),
                             (ssq_k, inv_k, scrk, scr2k)):
        nc.scalar.sqrt(sa[:], ssq[:])
        nc.vector.reciprocal_approx_accurate(inv[:], sa[:], sb[:])
        nc.vector.tensor_mul(sa[:], inv[:], inv[:])
        nc.vector.tensor_mul(sa[:], sa[:], ssq[:])
        nc.vector.tensor_scalar(sa[:], sa[:], -0.5, 1.5,
                                mybir.AluOpType.mult, mybir.AluOpType.add)
        nc.vector.tensor_mul(inv[:], inv[:], sa[:])

    # replicate inv norms to D partitions; normalize into bf16 K-padded tiles
    # (rep/invf shared across the q/k/den broadcasts; DMAs on sync only so the
    # scalar queue can't head-of-line block upcoming ACTIVATEs)
    rep = const.tile([VW, HW], F32)
    invf = const.tile([2, HW], F32)
    for r, (inv, srct, dst) in enumerate(((inv_q, qT, qTn), (inv_k, kT, kTn))):
        nc.sync.dma_start(invf[r:r + 1, :], inv[:])  # [NQB, QB] -> [1, HW]
        for p in range(D):
            nc.sync.dma_start(rep[p:p + 1, :], invf[r:r + 1, :])
        nc.vector.tensor_mul(dst[0:D, :], srct[:], rep[0:D, :])

    # ---------------- v prep (row layout, bf16, ones col) ----------------
    for c4 in range(NKC // 4):
        ps = ps_s.tile([KC, 4 * D], F32, tag="pss")
        for u in range(4):
            nc.tensor.matmul(ps[:, ts(u, D)], yTe[:, ts(4 * c4 + u, KC)],
                             wve[:], start=True, stop=True)
        for u in range(4):
            c = 4 * c4 + u
            nc.scalar.copy(vext[:, c * VW:c * VW + D], ps[:, ts(u, D)])

    # ---------------- main attention loop ----------------
    # oTe rows 0-7: unnormalized numerator; row 8: softmax denominator
    # (after scaling by the replicated reciprocal, row 8 becomes den/den = 1,
    # which is exactly the ones-row the output projection needs for be/8).
    oTe = const.tile([VW, HW], F32)
    den8 = const.tile([NQB, QB], F32)

    for j in range(NQB):
        po = ps_o.tile([VW, QB], F32, tag="pso")
        c = 0
        for gi, g in enumerate(GROUPS):
            ps = ps_s.tile([KC, GMAX * QB], F32, tag="pss")
            for u in range(g):
                nc.tensor.matmul(ps[:, ts(u, QB)], kTn[:, ts(c + u, KC)],
                                 qTn[:, ts(j, QB)], start=True, stop=True)
            es = expp.tile([KC, GMAX * QB], BF16, tag="es")
            nc.scalar.activation(es[:, 0:g * QB], ps[:, 0:g * QB],
                                 mybir.ActivationFunctionType.Exp)
            for u in range(g):
                cc = c + u
                nc.tensor.matmul(po[:], vext[:, cc * VW:(cc + 1) * VW],
                                 es[:, ts(u, QB)],
                                 start=(cc == 0), stop=(cc == NKC - 1))
            c += g
        nc.vector.tensor_copy(oTe[:, ts(j, QB)], po[:])

    # ---------------- normalize + output projection ----------------
    # repack denominator row [1, HW] -> [NQB, QB] via DMA (partition crossing)
    nc.sync.dma_start(den8[:], oTe[D:D + 1, :])
    invd, scr3 = scr2q, scrq  # prologue scratches, dead by now
    nc.vector.reciprocal_approx_accurate(invd[:], den8[:], scr3[:])
    nc.sync.dma_start(invf[0:1, :], invd[:])
    for p in range(VW):
        dmae[p % 2].dma_start(rep[p:p + 1, :], invf[0:1, :])
    # normalized oTe staged per block into the f32r sqq tile (rows 9.. still
    # zero) for a single-pass K-padded fp32r output projection
    resT = const.tile([C, HW], F32)
    for j in range(NQB):
        nc.vector.tensor_mul(sqq[0:VW, ts(j, QB)], oTe[:, ts(j, QB)],
                             rep[:, ts(j, QB)])
        ps = ps_s.tile([C, QB], F32, tag="pss")
        nc.tensor.matmul(ps[:], webe[:], sqq[:, ts(j, QB)], start=True,
                         stop=True)
        nc.vector.tensor_copy(resT[:, ts(j, QB)], ps[:])
        dmae[j % 2].dma_start(out_d[:, ts(j, QB)], resT[:, ts(j, QB)])


def _build():
    global _BUILT
    if _BUILT is not None:
        return _BUILT
    nc = bacc.Bacc("TRN2", target_bir_lowering=False, debug=False, num_devices=H)
    xTe_d = nc.dram_tensor("xTe", [KC, HW], F32R, kind="ExternalInput").ap()
    yTe_d = nc.dram_tensor("yTe", [KC, HW], F32R, kind="ExternalInput").ap()
    wqe_d = nc.dram_tensor("wqe", [KC, D], F32R, kind="ExternalInput").ap()
    wke_d = nc.dram_tensor("wke", [KC, D], F32R, kind="ExternalInput").ap()
    wve_d = nc.dram_tensor("wve", [KC, D], F32R, kind="ExternalInput").ap()
    webe_d = nc.dram_tensor("webe", [KC, C], F32R, kind="ExternalInput").ap()
    sel_d = nc.dram_tensor("sel", [KC, D * NQB], F32R, kind="ExternalInput").ap()
    out_d = nc.dram_tensor("resT", [C, HW], F32, kind="ExternalOutput").ap()
    with tile.TileContext(nc) as tc, ExitStack() as ctx:
        _body(ctx, tc, (xTe_d, yTe_d, wqe_d, wke_d, wve_d, webe_d, sel_d,
                        out_d[:]))
    nc.compile()
    _BUILT = nc
    return nc


def make_in_maps(x, y, Wq, bq, Wkv, bkv, We, be):
    x, y, Wq, bq, Wkv, bkv, We, be = (
        np.asarray(a, np.float32) for a in (x, y, Wq, bq, Wkv, bkv, We, be))
    ones = np.ones((1, HW), np.float32)
    zrows = np.zeros((KC - CE, HW), np.float32)
    xTe = np.ascontiguousarray(np.vstack([x[0].T, ones, zrows]))
    yTe = np.ascontiguousarray(np.vstack([y[0].T, ones, zrows]))
    sel = np.zeros((KC, D * NQB), np.float32)
    for j in range(NQB):
        sel[0:D, D * j + j] = 1.0
    zpad = np.zeros((KC - CE, D), np.float32)
    in_maps = []
    for h in range(H):
        sl = slice(h * D, (h + 1) * D)
        slv = slice(C + h * D, C + (h + 1) * D)
        in_maps.append({
            "xTe": xTe,
            "yTe": yTe,
            "wqe": np.ascontiguousarray(
                np.vstack([Wq[:, sl], bq[None, sl], zpad])),
            "wke": np.ascontiguousarray(
                np.vstack([Wkv[:, sl], bkv[None, sl], zpad])),
            "wve": np.ascontiguousarray(
                np.vstack([Wkv[:, slv], bkv[None, slv], zpad])),
            "webe": np.ascontiguousarray(np.vstack(
                [We[sl, :], be[None, :] / H, np.zeros((KC - VW, C), np.float32)])),
            "sel": sel,
        })
    return in_maps


def kernel(x, y, Wq, bq, Wkv, bkv, We, be):
    global LAST_RESULTS
    nc = _build()
    in_maps = make_in_maps(x, y, Wq, bq, Wkv, bkv, We, be)
    res = run_bass_kernel_spmd(nc, in_maps, core_ids=list(range(H)), trace=TRACE)
    LAST_RESULTS = res
    acc = np.zeros((C, HW), np.float64)
    for r in res.results:
        acc += r["resT"]
    return np.ascontiguousarray(acc.T[None]).astype(np.float32)


# revision 23
# speedup vs baseline: 1.0472x; 1.0472x over previous
"""Trainium2 Bass kernel for cross-attention (cosine-normalized, 8 heads).

Reference computation (full inputs x,y [1,4096,64]):
  q = x@Wq+bq ; k,v = split(y@Wkv+bkv) ; per head (8 heads, dim 8):
  attn = softmax(l2norm(q) @ l2norm(k)^T) ; out = attn@v
  result = concat_heads(out) @ We + be

Sharding: one head per NeuronCore (8 heads / 8 cores), SPMD program with
per-core weight slices. Each core returns resT_h = (out_h @ We_h + be/8)^T
as [64, 4096]; the host sums over cores and transposes.

Device algorithm per core (head h):
  - host passes xTe=[x^T; 1] [65,4096] (ones row folds biases into matmuls),
    yTe likewise, weight slices with bias rows, and a selector constant.
  - qT [8,4096] = Wqe^T @ xTe ; kT likewise (PE, K=65, fp32r).
  - norms in free layout: squares on DVE; selector matmuls pack per-block
    column sums into one [8,512] psum; invsqrt = newton(recip(sqrt)));
    flatten via DMA, replicate via gpsimd partition_broadcast; qT/kT
    normalized into fp32r tiles.
  - v chunks [128,8] = yTe_chunk^T @ Wve stored bf16 with ones column
    (vext [128, 9*32]); the ones column produces the softmax denominator.
  - main loop (8 q-blocks x 16 chunk-groups):
      scores^T [128,1024] = kTn_chunks^T qTn_block (fp32r, two matmuls)
      expS bf16 = Exp(scores) on ScalarE (cosine scores in [-1,1]: no
        max-subtraction needed)
      po [9,512] += vext_chunk^T @ expS  (rows 0-7 numerator, row 8 denom)
  - tail: invden = recip_accurate(den); oTe (incl. denom row) scaled by
    broadcast invden -> row 8 becomes exactly 1 = bias row for the final
    fp32 projection resT = WeBe^T @ oTe; DMA out.
"""

import sys

import numpy as np

for _p in ("/opt/trn_rl_repo",):
    if _p not in sys.path:
        sys.path.insert(0, _p)

from contextlib import ExitStack

import concourse.bass as bass
import concourse.tile as tile
from concourse import bacc, mybir
from concourse.bass import ts
from concourse.bass_utils import run_bass_kernel_spmd

F32 = mybir.dt.float32
F32R = mybir.dt.float32r
BF16 = mybir.dt.bfloat16

HW = 4096          # sequence length
C = 64             # model dim
H = 8              # heads
D = 8              # head dim
CE = C + 1         # +ones row for bias folding
QB = 512           # q block
NQB = HW // QB     # 8
KC = 128           # k chunk
NKC = HW // KC     # 32
GROUPS = [3] * 10 + [2]   # k-chunks per exp/ACT group (32 total)
GMAX = max(GROUPS)
VW = D + 1         # v + ones column

REPL = "dma"        # inv replication: gpsimd partition_broadcast vs row DMAs

_BUILT = None
TRACE = False
LAST_RESULTS = None


def _body(ctx, tc, dram):
    nc = tc.nc
    xTe_d, yTe_d, wqe_d, wke_d, wve_d, webe_d, sel_d, out_d = dram
    U32 = mybir.dt.uint32

    const = ctx.enter_context(tc.tile_pool(name="const", bufs=1))
    expp = ctx.enter_context(tc.tile_pool(name="exps", bufs=3))
    ps_s = ctx.enter_context(tc.tile_pool(name="ps_s", bufs=2, space="PSUM"))
    ps_o = ctx.enter_context(tc.tile_pool(name="ps_o", bufs=2, space="PSUM"))

    # K-padded tiles: rows beyond the live ones are zeroed so every matmul
    # contracts over K=128 (K<=32 matmuls run at half the PE clock - measured).
    # All zeroing on gpsimd to keep the DVE free for the norm chain.
    xTe = const.tile([KC, HW], F32R)   # host-padded: rows CE..127 zero
    yTe = const.tile([KC, HW], F32R)
    sqq = const.tile([KC, HW], F32R)   # q squares scratch / f32r oT staging
    sqk = const.tile([KC, HW], F32R)   # k squares scratch
    qTn = const.tile([KC, HW], BF16)
    kTn = const.tile([KC, HW], BF16)
    vext = const.tile([KC, VW * NKC], BF16)
    nc.gpsimd.memset(sqq[:].bitcast(U32), 0)
    nc.gpsimd.memset(sqk[:].bitcast(U32), 0)
    nc.gpsimd.memset(qTn[:].bitcast(mybir.dt.uint16), 0)
    nc.gpsimd.memset(kTn[:].bitcast(mybir.dt.uint16), 0)
    nc.gpsimd.memset(vext[:], 1.0)

    # ---------------- loads ----------------
    wqe = const.tile([KC, D], F32R)
    nc.sync.dma_start(wqe[:], wqe_d)
    wke = const.tile([KC, D], F32R)
    nc.sync.dma_start(wke[:], wke_d)
    wve = const.tile([KC, D], F32R)
    nc.sync.dma_start(wve[:], wve_d)
    webe = const.tile([KC, C], F32R)
    nc.scalar.dma_start(webe[:], webe_d)
    sel = const.tile([KC, D * NQB], F32R)
    nc.scalar.dma_start(sel[:], sel_d)
    dmae = [nc.sync, nc.scalar]
    for j in range(NQB):
        dmae[j % 2].dma_start(xTe[:, ts(j, QB)], xTe_d[:, ts(j, QB)])
        dmae[(j + 1) % 2].dma_start(yTe[:, ts(j, QB)], yTe_d[:, ts(j, QB)])

    # ---------------- q/k projections (transposed layout) ----------------
    qT = const.tile([D, HW], F32)
    kT = const.tile([D, HW], F32)
    for dst, w, srct, eng in ((qT, wqe, xTe, nc.scalar), (kT, wke, yTe, None)):
        for j in range(NQB):
            ps = ps_s.tile([D, QB], F32, tag="pss")
            nc.tensor.matmul(ps[:], w[:], srct[:, ts(j, QB)], start=True,
                             stop=True)
            if eng is nc.scalar:
                nc.scalar.copy(dst[:, ts(j, QB)], ps[:])
            else:
                nc.vector.tensor_copy(dst[:, ts(j, QB)], ps[:])

    # ---------------- inverse norms (free layout, partition-packed) -------
    ssq_q = const.tile([NQB, QB], F32)
    ssq_k = const.tile([NQB, QB], F32)
    for srct, ssq, sq in ((qT, ssq_q, sqq), (kT, ssq_k, sqk)):
        if sq is sqq:
            nc.scalar.square(sq[0:D, :], srct[:])
        else:
            nc.vector.tensor_mul(sq[0:D, :], srct[:], srct[:])
        ps = ps_s.tile([NQB, QB], F32, tag="pss")
        for j in range(NQB):
            nc.tensor.matmul(ps[:], sel[:, ts(j, D)], sq[:, ts(j, QB)],
                             start=(j == 0), stop=(j == NQB - 1))
        nc.vector.tensor_copy(ssq[:], ps[:])

    # invsqrt = newton(recip_accurate(sqrt(ssq))); separate scratches so the
    # q and k chains interleave
    inv_q = const.tile([NQB, QB], F32)
    inv_k = const.tile([NQB, QB], F32)
    scrq = const.tile([NQB, QB], F32)
    scr2q = const.tile([NQB, QB], F32)
    scrk = const.tile([NQB, QB], F32)
    scr2k = const.tile([NQB, QB], F32)
    for ssq, inv, sa, sb in ((ssq_q, inv_q, scrq, scr2q),
                             (ssq_k, inv_k, scrk, scr2k)):
        nc.scalar.sqrt(sa[:], ssq[:])
        nc.vector.reciprocal_approx_accurate(inv[:], sa[:], sb[:])
        nc.vector.tensor_mul(sa[:], inv[:], inv[:])
        nc.vector.tensor_mul(sa[:], sa[:], ssq[:])
        nc.vector.tensor_scalar(sa[:], sa[:], -0.5, 1.5,
                                mybir.AluOpType.mult, mybir.AluOpType.add)
        nc.vector.tensor_mul(inv[:], inv[:], sa[:])

    # replicate inv norms to D partitions; normalize into bf16 K-padded tiles.
    # q uses rep; k reuses the dead xTe tile so the two chains don't WAR-
    # serialize on one broadcast buffer.
    rep = const.tile([VW, HW], F32)
    invf = const.tile([2, HW], F32)
    rep_k = xTe[0:VW, :].bitcast(F32)
    for r, (inv, srct, dst, rp) in enumerate(
            ((inv_q, qT, qTn, rep[:]), (inv_k, kT, kTn, rep_k))):
        nc.sync.dma_start(invf[r:r + 1, :], inv[:])  # [NQB, QB] -> [1, HW]
        for p in range(D):
            dmae[p % 2].dma_start(rp[p:p + 1, :], invf[r:r + 1, :])
        nc.vector.tensor_mul(dst[0:D, :], srct[:], rp[0:D, :])

    # ---------------- v prep (row layout, bf16, ones col) ----------------
    for c4 in range(NKC // 4):
        ps = ps_s.tile([KC, 4 * D], F32, tag="pss")
        for u in range(4):
            nc.tensor.matmul(ps[:, ts(u, D)], yTe[:, ts(4 * c4 + u, KC)],
                             wve[:], start=True, stop=True)
        for u in range(4):
            c = 4 * c4 + u
            nc.scalar.copy(vext[:, c * VW:c * VW + D], ps[:, ts(u, D)])

    # ---------------- main attention loop ----------------
    # oTe rows 0-7: unnormalized numerator; row 8: softmax denominator
    # (after scaling by the replicated reciprocal, row 8 becomes den/den = 1,
    # which is exactly the ones-row the output projection needs for be/8).
    oTe = const.tile([VW, HW], F32)
    den8 = const.tile([NQB, QB], F32)

    for j in range(NQB):
        po = ps_o.tile([VW, QB], F32, tag="pso")
        c = 0
        for gi, g in enumerate(GROUPS):
            ps = ps_s.tile([KC, GMAX * QB], F32, tag="pss")
            for u in range(g):
                nc.tensor.matmul(ps[:, ts(u, QB)], kTn[:, ts(c + u, KC)],
                                 qTn[:, ts(j, QB)], start=True, stop=True)
            es = expp.tile([KC, GMAX * QB], BF16, tag="es")
            nc.scalar.activation(es[:, 0:g * QB], ps[:, 0:g * QB],
                                 mybir.ActivationFunctionType.Exp)
            for u in range(g):
                cc = c + u
                nc.tensor.matmul(po[:], vext[:, cc * VW:(cc + 1) * VW],
                                 es[:, ts(u, QB)],
                                 start=(cc == 0), stop=(cc == NKC - 1))
            c += g
        nc.vector.tensor_copy(oTe[:, ts(j, QB)], po[:])

    # ---------------- normalize + output projection ----------------
    # repack denominator row [1, HW] -> [NQB, QB] via DMA (partition crossing)
    nc.sync.dma_start(den8[:], oTe[D:D + 1, :])
    invd, scr3 = scr2q, scrq  # prologue scratches, dead by now
    nc.vector.reciprocal_approx_accurate(invd[:], den8[:], scr3[:])
    nc.sync.dma_start(invf[0:1, :], invd[:])
    for p in range(VW):
        dmae[p % 2].dma_start(rep[p:p + 1, :], invf[0:1, :])
    # normalized oTe staged per block into the f32r sqq tile (rows 9.. still
    # zero) for a single-pass K-padded fp32r output projection
    resT = const.tile([C, HW], F32)
    for j in range(NQB):
        nc.vector.tensor_mul(sqq[0:VW, ts(j, QB)], oTe[:, ts(j, QB)],
                             rep[:, ts(j, QB)])
        ps = ps_s.tile([C, QB], F32, tag="pss")
        nc.tensor.matmul(ps[:], webe[:], sqq[:, ts(j, QB)], start=True,
                         stop=True)
        nc.vector.tensor_copy(resT[:, ts(j, QB)], ps[:])
        dmae[j % 2].dma_start(out_d[:, ts(j, QB)], resT[:, ts(j, QB)])


def _build():
    global _BUILT
    if _BUILT is not None:
        return _BUILT
    nc = bacc.Bacc("TRN2", target_bir_lowering=False, debug=False, num_devices=H)
    xTe_d = nc.dram_tensor("xTe", [KC, HW], F32R, kind="ExternalInput").ap()
    yTe_d = nc.dram_tensor("yTe", [KC, HW], F32R, kind="ExternalInput").ap()
    wqe_d = nc.dram_tensor("wqe", [KC, D], F32R, kind="ExternalInput").ap()
    wke_d = nc.dram_tensor("wke", [KC, D], F32R, kind="ExternalInput").ap()
    wve_d = nc.dram_tensor("wve", [KC, D], F32R, kind="ExternalInput").ap()
    webe_d = nc.dram_tensor("webe", [KC, C], F32R, kind="ExternalInput").ap()
    sel_d = nc.dram_tensor("sel", [KC, D * NQB], F32R, kind="ExternalInput").ap()
    out_d = nc.dram_tensor("resT", [C, HW], F32, kind="ExternalOutput").ap()
    with tile.TileContext(nc) as tc, ExitStack() as ctx:
        _body(ctx, tc, (xTe_d, yTe_d, wqe_d, wke_d, wve_d, webe_d, sel_d,
                        out_d[:]))
    nc.compile()
    _BUILT = nc
    return nc


def make_in_maps(x, y, Wq, bq, Wkv, bkv, We, be):
    x, y, Wq, bq, Wkv, bkv, We, be = (
        np.asarray(a, np.float32) for a in (x, y, Wq, bq, Wkv, bkv, We, be))
    ones = np.ones((1, HW), np.float32)
    zrows = np.zeros((KC - CE, HW), np.float32)
    xTe = np.ascontiguousarray(np.vstack([x[0].T, ones, zrows]))
    yTe = np.ascontiguousarray(np.vstack([y[0].T, ones, zrows]))
    sel = np.zeros((KC, D * NQB), np.float32)
    for j in range(NQB):
        sel[0:D, D * j + j] = 1.0
    zpad = np.zeros((KC - CE, D), np.float32)
    in_maps = []
    for h in range(H):
        sl = slice(h * D, (h + 1) * D)
        slv = slice(C + h * D, C + (h + 1) * D)
        in_maps.append({
            "xTe": xTe,
            "yTe": yTe,
            "wqe": np.ascontiguousarray(
                np.vstack([Wq[:, sl], bq[None, sl], zpad])),
            "wke": np.ascontiguousarray(
                np.vstack([Wkv[:, sl], bkv[None, sl], zpad])),
            "wve": np.ascontiguousarray(
                np.vstack([Wkv[:, slv], bkv[None, slv], zpad])),
            "webe": np.ascontiguousarray(np.vstack(
                [We[sl, :], be[None, :] / H, np.zeros((KC - VW, C), np.float32)])),
            "sel": sel,
        })
    return in_maps


def kernel(x, y, Wq, bq, Wkv, bkv, We, be):
    global LAST_RESULTS
    nc = _build()
    in_maps = make_in_maps(x, y, Wq, bq, Wkv, bkv, We, be)
    res = run_bass_kernel_spmd(nc, in_maps, core_ids=list(range(H)), trace=TRACE)
    LAST_RESULTS = res
    acc = np.zeros((C, HW), np.float64)
    for r in res.results:
        acc += r["resT"]
    return np.ascontiguousarray(acc.T[None]).astype(np.float32)
